# revision 62
# baseline (speedup 1.0000x reference)
"""DEC soft-assignment (student-t, row-normalized) Trainium2 Bass kernel.

q[n,k] = (1 + ||x_n - c_k||^2/alpha)^(-(alpha+1)/2), row-normalized.

Strategy (8 cores, data-parallel over N):
  ||x-c||^2 = ||x||^2 - 2 x.c + ||c||^2 expanded on-chip.
  - cenT [d,k] built once via PE transposes, scaled by -2/alpha, stored fp8e4
    (cross term only; x^2 / c^2 terms stay fp32 so the quantization error is
    limited to ~0.3% of d^2).
  - csq row built on-chip (PE transposes of csq columns), carries the +1.
  - 512-row supertiles (4 x 128-row blocks) amortize DMA trigger cost.
  - per block: PE-transposes emb (fp32) into a 2-block PSUM tile; one
    copy-convert to fp8e4 SBUF per block-PAIR (alternating ACT/DVE); PE
    accumulates csq fold (f32r) + 3 DoubleRow fp8 matmuls (0.5 cyc/row);
    ONE ACT Reciprocal activation computes numer = 1/(psum + xsq[p]) AND
    the row-sum via accum_out; DVE does xsq, batched 1/rowsum, normalize.
"""

import contextlib
import os
import sys

sys.path.insert(0, "/opt/trn_rl_repo")

import numpy as np

N_CORES = 8
N, D, K = 65536, 768, 512
NC_ROWS = N // N_CORES          # 8192 rows per core
P = 128                         # partitions
S_BLK = 4                       # 128-row blocks per supertile
S_ROWS = P * S_BLK              # 512 rows per supertile
N_SUPER = NC_ROWS // S_ROWS     # 16 supertiles per core
D_CHUNKS = D // P               # 6 contraction chunks

_CACHE = {}


def _emit(nc, tc, emb_d, cen_d, out_d, alpha: float, n_rows: int):
    """Emit the per-core program into an open TileContext."""
    import concourse.bass as bass
    import concourse.mybir as mybir
    from concourse.masks import make_identity

    f32 = mybir.dt.float32
    f32r = mybir.dt.float32r
    fp8 = mybir.dt.float8e4
    ts = bass.ts
    DR = mybir.MatmulPerfMode.DoubleRow

    inv_a = 1.0 / alpha
    power = (alpha + 1.0) / 2.0
    n_super = n_rows // S_ROWS
    reps = int(os.environ.get("KBENCH_REPS", "1"))

    emb_v = emb_d.rearrange("(s a p) d -> s p a d", p=P, a=S_BLK)
    out_v = out_d.rearrange("(s a p) k -> s p a k", p=P, a=S_BLK)

    def act_recip(out_ap, in_ap, bias_ap, accum_ap):
        """numer = 1/(in + bias[p]); accum_out = row-sum(numer).
        Emits InstActivation(Reciprocal) directly: measured max rel err
        ~1.2e-5 on hw, well within this problem's 2e-2 gate."""
        eng = nc.scalar
        ins = [
            eng.lower_ap(in_ap),
            eng.lower_ap(bias_ap),
            mybir.ImmediateValue(dtype=f32, value=1.0),
            mybir.ImmediateValue(dtype=f32, value=0.0),
        ]
        outs = [eng.lower_ap(out_ap), eng.lower_ap(accum_ap)]
        return eng.add_instruction(
            mybir.InstActivation(
                name=nc.get_next_instruction_name(),
                func=mybir.ActivationFunctionType.Reciprocal,
                ins=ins,
                outs=outs,
            )
        )

    with contextlib.ExitStack() as stack:
        const_pool = stack.enter_context(tc.tile_pool(name="const", bufs=1))
        cen_pool = stack.enter_context(tc.tile_pool(name="cent", bufs=1))
        in_pool = stack.enter_context(tc.tile_pool(name="io_in", bufs=4))
        # prefetch the first supertiles' embeddings during setup
        prefetched = {}
        for i in range(min(2, n_super)):
            t_in = in_pool.tile([P, S_BLK, D], f32, tag="emb")
            nc.sync.dma_start(t_in[:], emb_v[i])
            prefetched[i] = t_in
        with (
            tc.tile_pool(name="setup", bufs=2) as setup_pool,
            tc.tile_pool(name="setup_ps", bufs=2, space=bass.MemorySpace.PSUM) as setup_ps,
            tc.tile_pool(name="setup_ps1", bufs=1, space=bass.MemorySpace.PSUM) as setup_ps1,
        ):
            identity = const_pool.tile([P, P], f32)
            make_identity(nc, identity[:])
            # f32r identity: transpose cost keys on the moving operand (the
            # identity); f32r runs 1.5 cyc/row vs fp32's 2.0. (fp8/bf16 would
            # be 1.0 but mixing 32-bit with sub-32-bit operands is rejected.)
            id_r = const_pool.tile([P, P], f32r)
            nc.scalar.copy(id_r[:], identity[:])
            ones_row_f32 = const_pool.tile([1, P], f32)
            nc.gpsimd.memset(ones_row_f32[:], 1.0)
            ones_row = const_pool.tile([1, P], f32r)
            nc.scalar.copy(ones_row[:], ones_row_f32[:])

            # cenT [d=128, chunk, k=512] scaled by -2/alpha, fp8e4, resident.
            cenT = cen_pool.tile([P, D_CHUNKS, K], fp8, name="cenT")
            csq_cols = const_pool.tile([P, K // P], f32)
            # full-width fold operands: ones_mat [128,128] stationary and
            # csq_bcast [128,K] moving = (1+csq/a)/128 on every partition.
            # (A [1,K] moving operand throttles the PE on single-partition
            # SBUF reads; broadcasting restores full stream rate.)
            ones_mat_f = const_pool.tile([P, P], f32)
            ones_mat = const_pool.tile([P, P], f32r)
            csq_sc = const_pool.tile([1, K], f32r)
            csq_bcast = const_pool.tile([P, K], f32r)
            nc.gpsimd.memset(ones_mat_f[:], 1.0)
            nc.scalar.copy(ones_mat[:], ones_mat_f[:])

            # PE warmup: a dense burst of dummy transposes while the first
            # DMAs land, ramping the tensor engine's p-state to full clock
            # before real work arrives.
            n_warm = int(os.environ.get("KOPT_WARM", "28"))
            if n_warm:
                warm_ps = setup_ps1.tile([P, P], f32, tag="warm")
                for _ in range(n_warm):
                    nc.tensor.transpose(warm_ps[:], identity[:], identity[:])

            csq_ps = setup_ps1.tile([P, K], f32, tag="csq_ps")
            for t in range(K // P):
                cnat = setup_pool.tile([P, D], f32, tag="cnat")
                nc.sync.dma_start(cnat[:], cen_d[ts(t, P), :])
                # csq for this block of 128 clusters (scaled by 1/alpha)
                scr = setup_pool.tile([P, D], f32, tag="cscr")
                nc.vector.scalar_tensor_tensor(
                    out=scr[:],
                    in0=cnat[:],
                    scalar=inv_a,
                    in1=cnat[:],
                    op0=mybir.AluOpType.mult,
                    op1=mybir.AluOpType.mult,
                    accum_out=csq_cols[:, t : t + 1],
                )
                # transpose the 6 chunks of this block, convert to fp8
                tps = setup_ps.tile([P, D], f32, tag="tps")
                for j in range(D_CHUNKS):
                    nc.tensor.transpose(
                        tps[:, ts(j, P)], cnat[:, ts(j, P)], identity[:]
                    )
                nc.scalar.mul(
                    cenT[:, :, ts(t, P)],
                    tps[:].rearrange("p (c b) -> p c b", c=D_CHUNKS),
                    -2.0 * inv_a,
                )
                # csq column -> row segment via PE transpose (on-chip)
                nc.tensor.transpose(
                    csq_ps[0:1, ts(t, P)], csq_cols[:, t : t + 1], identity[:]
                )
            # csq_sc = (1 + csq/alpha)/128, then broadcast to 128 partitions
            # via a one-off matmul (slow single-partition moving is fine once).
            invp = const_pool.tile([1, 1], f32)
            nc.gpsimd.memset(invp[:], 1.0 / P)
            nc.scalar.activation(
                csq_sc[:],
                csq_ps[0:1, :],
                mybir.ActivationFunctionType.Identity,
                bias=invp[:, 0:1],
                scale=1.0 / P,
            )
            bc_ps = setup_ps1.tile([P, K], f32, tag="bc_ps")
            nc.tensor.matmul(
                bc_ps[:], ones_row[:], csq_sc[:], start=True, stop=True
            )
            nc.scalar.copy(csq_bcast[:], bc_ps[:])

        with (
            tc.tile_pool(name="work", bufs=3) as work_pool,
            tc.tile_pool(name="blk", bufs=4) as blk_pool,
            tc.tile_pool(name="io_out", bufs=3) as out_pool,
            tc.tile_pool(name="tp_ps", bufs=2, space=bass.MemorySpace.PSUM) as tp_ps,
            tc.tile_pool(name="mm_ps", bufs=4, space=bass.MemorySpace.PSUM) as mm_ps,
        ):
            for i in [t for _ in range(reps) for t in range(n_super)]:
                emb_nat = prefetched.pop(i, None)
                if emb_nat is None:
                    emb_nat = in_pool.tile([P, S_BLK, D], f32, tag="emb")
                    nc.sync.dma_start(emb_nat[:], emb_v[i])

                embT = work_pool.tile(
                    [P, S_BLK * D_CHUNKS, P], fp8, tag="embT"
                )
                xsq = work_pool.tile([P, S_BLK], f32, tag="xsq")
                numer = work_pool.tile([P, S_BLK, K], f32, tag="numer")
                out_t = out_pool.tile([P, S_BLK, K], f32, tag="out")

                for a in range(S_BLK):
                    # xsq = ||x||^2/alpha; one block on ACT, rest on DVE
                    sq_scr = blk_pool.tile([P, D], f32, tag="sqscr")
                    if a == 0:
                        nc.scalar.activation(
                            sq_scr[:],
                            emb_nat[:, a, :],
                            mybir.ActivationFunctionType.Square,
                            scale=float(inv_a**0.5),
                            accum_out=xsq[:, a : a + 1],
                        )
                    else:
                        nc.vector.scalar_tensor_tensor(
                            out=sq_scr[:],
                            in0=emb_nat[:, a, :],
                            scalar=inv_a,
                            in1=emb_nat[:, a, :],
                            op0=mybir.AluOpType.mult,
                            op1=mybir.AluOpType.mult,
                            accum_out=xsq[:, a : a + 1],
                        )
                    dst = embT[:, a * D_CHUNKS : (a + 1) * D_CHUNKS, :]
                    tps = tp_ps.tile([P, D_CHUNKS, P], f32, tag="tps")
                    for j in range(D_CHUNKS):
                        nc.tensor.transpose(
                            tps[:, j, :], emb_nat[:, a, ts(j, P)], identity[:]
                        )
                    if a % 2 == 0:
                        nc.scalar.copy(dst, tps[:])
                    else:
                        nc.vector.tensor_copy(out=dst, in_=tps[:])

                for a in range(S_BLK):
                    # PSUM <- (1 + csq/a) - (2/a) x.c
                    ps = mm_ps.tile([P, K], f32, tag="cross")
                    nc.tensor.matmul(
                        ps[:],
                        ones_mat[:],
                        csq_bcast[:],
                        start=True,
                        stop=False,
                    )
                    for j in range(0, D_CHUNKS, 2):
                        nc.tensor.matmul(
                            ps[:],
                            embT[:, a * D_CHUNKS + j : a * D_CHUNKS + j + 2, :],
                            cenT[:, j : j + 2, :],
                            start=False,
                            stop=(j == D_CHUNKS - 2),
                            perf_mode=DR,
                        )

                    if power == 1.0:
                        # numer = 1/(ps + xsq) and rowsum in ONE ACT op
                        rs_a = blk_pool.tile([P, 1], f32, tag="rs")
                        act_recip(
                            numer[:, a, :],
                            ps[:],
                            xsq[:, a : a + 1],
                            rs_a[:],
                        )
                        inv_a_t = blk_pool.tile([P, 1], f32, tag="inv")
                        nc.vector.reciprocal(inv_a_t[:], rs_a[:])
                        nc.vector.tensor_scalar_mul(
                            out_t[:, a, :], numer[:, a, :], inv_a_t[:]
                        )
                    else:
                        denom = blk_pool.tile([P, K], f32, tag="denom")
                        nc.scalar.activation(
                            denom[:],
                            ps[:],
                            mybir.ActivationFunctionType.Identity,
                            bias=xsq[:, a : a + 1],
                            scale=1.0,
                        )
                        lnd = blk_pool.tile([P, K], f32, tag="lnd")
                        rs_a = blk_pool.tile([P, 1], f32, tag="rs")
                        nc.scalar.activation(
                            lnd[:], denom[:], mybir.ActivationFunctionType.Ln
                        )
                        nc.scalar.activation(
                            numer[:, a, :],
                            lnd[:],
                            mybir.ActivationFunctionType.Exp,
                            scale=-power,
                            accum_out=rs_a[:],
                        )
                        inv_a_t = blk_pool.tile([P, 1], f32, tag="inv")
                        nc.vector.reciprocal(inv_a_t[:], rs_a[:])
                        nc.vector.tensor_scalar_mul(
                            out_t[:, a, :], numer[:, a, :], inv_a_t[:]
                        )

                # output trigger on the (otherwise idle) GpSimd queue
                nc.gpsimd.dma_start(out_v[i], out_t[:])


def _build_program(alpha: float):
    """Standalone Bacc program (for CoreSim checks / bench2)."""
    import concourse.bacc as bacc
    import concourse.mybir as mybir
    import concourse.tile as tile

    f32 = mybir.dt.float32
    nc = bacc.Bacc(None, target_bir_lowering=False, debug=False, num_devices=N_CORES)
    emb_d = nc.declare_dram_parameter("embeddings", [NC_ROWS, D], f32, isOutput=False)
    cen_d = nc.declare_dram_parameter("cluster_centers", [K, D], f32, isOutput=False)
    out_d = nc.declare_dram_parameter("cluster_p", [NC_ROWS, K], f32, isOutput=True)
    with tile.TileContext(nc) as tc:
        _emit(nc, tc, emb_d, cen_d, out_d, alpha, NC_ROWS)
    nc.finalize()
    return nc


def _get_jitted(alpha: float):
    key = (
        float(alpha),
        os.environ.get("KBENCH_REPS", "1"),
        os.environ.get("KOPT_TPS", "pair"),
        os.environ.get("KOPT_RECIP", "fused"),
    )
    if key in _CACHE:
        return _CACHE[key]

    import jax
    from jax.experimental.shard_map import shard_map
    from jax.sharding import Mesh, PartitionSpec as PS

    import concourse.mybir as mybir
    import concourse.tile as tile
    from concourse.bass2jax import bass_jit

    f32 = mybir.dt.float32

    def body(nc, emb, cen):
        out_d = nc.dram_tensor(
            "cluster_p", [NC_ROWS, K], f32, kind="ExternalOutput"
        )
        with tile.TileContext(nc) as tc:
            _emit(nc, tc, emb, cen, out_d, float(alpha), NC_ROWS)
        return out_d

    f = bass_jit(body, num_devices=N_CORES)
    mesh = Mesh(np.asarray(jax.devices()[:N_CORES]), ("core",))
    sharded = shard_map(
        f,
        mesh=mesh,
        in_specs=(PS("core"), PS(None)),
        out_specs=PS("core"),
        check_rep=False,
    )
    jitted = jax.jit(sharded)
    _CACHE[key] = (jitted, mesh)
    return _CACHE[key]


def kernel(embeddings, cluster_centers, alpha):
    emb = np.ascontiguousarray(np.asarray(embeddings, dtype=np.float32))
    cen = np.ascontiguousarray(np.asarray(cluster_centers, dtype=np.float32))
    jitted, _ = _get_jitted(float(alpha))
    try:
        out = jitted(emb, cen)
        return np.asarray(out)
    except Exception:
        # transient device hiccups have been observed; retry once
        import time as _time

        _time.sleep(60)
        out = jitted(emb, cen)
        return np.asarray(out)


# revision 63
# speedup vs baseline: 1.0280x; 1.0280x over previous
"""DEC soft-assignment (student-t, row-normalized) Trainium2 Bass kernel.

q[n,k] = (1 + ||x_n - c_k||^2/alpha)^(-(alpha+1)/2), row-normalized.

Strategy (8 cores, data-parallel over N):
  ||x-c||^2 = ||x||^2 - 2 x.c + ||c||^2 expanded on-chip.
  - cenT [d,k] built once via PE transposes, scaled by -2/alpha, stored fp8e4
    (cross term only; x^2 / c^2 terms stay fp32 so the quantization error is
    limited to ~0.3% of d^2).
  - csq row built on-chip (PE transposes of csq columns), carries the +1.
  - 512-row supertiles (4 x 128-row blocks) amortize DMA trigger cost.
  - per block: PE-transposes emb (fp32) into a 2-block PSUM tile; one
    copy-convert to fp8e4 SBUF per block-PAIR (alternating ACT/DVE); PE
    accumulates csq fold (f32r) + 3 DoubleRow fp8 matmuls (0.5 cyc/row);
    ONE ACT Reciprocal activation computes numer = 1/(psum + xsq[p]) AND
    the row-sum via accum_out; DVE does xsq, batched 1/rowsum, normalize.
"""

import contextlib
import os
import sys

sys.path.insert(0, "/opt/trn_rl_repo")

import numpy as np

N_CORES = 8
N, D, K = 65536, 768, 512
NC_ROWS = N // N_CORES          # 8192 rows per core
P = 128                         # partitions
S_BLK = 4                       # 128-row blocks per supertile
S_ROWS = P * S_BLK              # 512 rows per supertile
N_SUPER = NC_ROWS // S_ROWS     # 16 supertiles per core
D_CHUNKS = D // P               # 6 contraction chunks

_CACHE = {}


def _emit(nc, tc, emb_d, cen_d, out_d, alpha: float, n_rows: int):
    """Emit the per-core program into an open TileContext."""
    import concourse.bass as bass
    import concourse.mybir as mybir
    from concourse.masks import make_identity

    f32 = mybir.dt.float32
    f32r = mybir.dt.float32r
    fp8 = mybir.dt.float8e4
    ts = bass.ts
    DR = mybir.MatmulPerfMode.DoubleRow

    inv_a = 1.0 / alpha
    power = (alpha + 1.0) / 2.0
    n_super = n_rows // S_ROWS
    reps = int(os.environ.get("KBENCH_REPS", "1"))

    emb_v = emb_d.rearrange("(s a p) d -> s p a d", p=P, a=S_BLK)
    out_v = out_d.rearrange("(s a p) k -> s p a k", p=P, a=S_BLK)

    def act_recip(out_ap, in_ap, bias_ap, accum_ap):
        """numer = 1/(in + bias[p]); accum_out = row-sum(numer).
        Emits InstActivation(Reciprocal) directly: measured max rel err
        ~1.2e-5 on hw, well within this problem's 2e-2 gate."""
        eng = nc.scalar
        ins = [
            eng.lower_ap(in_ap),
            eng.lower_ap(bias_ap),
            mybir.ImmediateValue(dtype=f32, value=1.0),
            mybir.ImmediateValue(dtype=f32, value=0.0),
        ]
        outs = [eng.lower_ap(out_ap), eng.lower_ap(accum_ap)]
        return eng.add_instruction(
            mybir.InstActivation(
                name=nc.get_next_instruction_name(),
                func=mybir.ActivationFunctionType.Reciprocal,
                ins=ins,
                outs=outs,
            )
        )

    with contextlib.ExitStack() as stack:
        const_pool = stack.enter_context(tc.tile_pool(name="const", bufs=1))
        cen_pool = stack.enter_context(tc.tile_pool(name="cent", bufs=1))
        in_pool = stack.enter_context(tc.tile_pool(name="io_in", bufs=5))
        # prefetch the first supertiles' embeddings during setup
        prefetched = {}
        for i in range(min(3, n_super)):
            t_in = in_pool.tile([P, S_BLK, D], f32, tag="emb")
            nc.sync.dma_start(t_in[:], emb_v[i])
            prefetched[i] = t_in
        with (
            tc.tile_pool(name="setup", bufs=2) as setup_pool,
            tc.tile_pool(name="setup_ps", bufs=2, space=bass.MemorySpace.PSUM) as setup_ps,
            tc.tile_pool(name="setup_ps1", bufs=1, space=bass.MemorySpace.PSUM) as setup_ps1,
        ):
            identity = const_pool.tile([P, P], f32)
            make_identity(nc, identity[:])
            # f32r identity: transpose cost keys on the moving operand (the
            # identity); f32r runs 1.5 cyc/row vs fp32's 2.0. (fp8/bf16 would
            # be 1.0 but mixing 32-bit with sub-32-bit operands is rejected.)
            id_r = const_pool.tile([P, P], f32r)
            nc.scalar.copy(id_r[:], identity[:])
            ones_row_f32 = const_pool.tile([1, P], f32)
            nc.gpsimd.memset(ones_row_f32[:], 1.0)
            ones_row = const_pool.tile([1, P], f32r)
            nc.scalar.copy(ones_row[:], ones_row_f32[:])

            # cenT [d=128, chunk, k=512] scaled by -2/alpha, fp8e4, resident.
            cenT = cen_pool.tile([P, D_CHUNKS, K], fp8, name="cenT")
            csq_cols = const_pool.tile([P, K // P], f32)
            # full-width fold operands: ones_mat [128,128] stationary and
            # csq_bcast [128,K] moving = (1+csq/a)/128 on every partition.
            # (A [1,K] moving operand throttles the PE on single-partition
            # SBUF reads; broadcasting restores full stream rate.)
            ones_mat_f = const_pool.tile([P, P], f32)
            ones_mat = const_pool.tile([P, P], f32r)
            csq_sc = const_pool.tile([1, K], f32r)
            csq_bcast = const_pool.tile([P, K], f32r)
            nc.gpsimd.memset(ones_mat_f[:], 1.0)
            nc.scalar.copy(ones_mat[:], ones_mat_f[:])

            # PE warmup: a dense burst of dummy transposes while the first
            # DMAs land, ramping the tensor engine's p-state to full clock
            # before real work arrives.
            n_warm = int(os.environ.get("KOPT_WARM", "28"))
            if n_warm:
                warm_ps = setup_ps1.tile([P, P], f32, tag="warm")
                for _ in range(n_warm):
                    nc.tensor.transpose(warm_ps[:], identity[:], identity[:])

            csq_ps = setup_ps1.tile([P, K], f32, tag="csq_ps")
            for t in range(K // P):
                cnat = setup_pool.tile([P, D], f32, tag="cnat")
                nc.sync.dma_start(cnat[:], cen_d[ts(t, P), :])
                # csq for this block of 128 clusters (scaled by 1/alpha)
                scr = setup_pool.tile([P, D], f32, tag="cscr")
                nc.vector.scalar_tensor_tensor(
                    out=scr[:],
                    in0=cnat[:],
                    scalar=inv_a,
                    in1=cnat[:],
                    op0=mybir.AluOpType.mult,
                    op1=mybir.AluOpType.mult,
                    accum_out=csq_cols[:, t : t + 1],
                )
                # transpose the 6 chunks of this block, convert to fp8
                tps = setup_ps.tile([P, D], f32, tag="tps")
                for j in range(D_CHUNKS):
                    nc.tensor.transpose(
                        tps[:, ts(j, P)], cnat[:, ts(j, P)], identity[:]
                    )
                nc.scalar.mul(
                    cenT[:, :, ts(t, P)],
                    tps[:].rearrange("p (c b) -> p c b", c=D_CHUNKS),
                    -2.0 * inv_a,
                )
                # csq column -> row segment via PE transpose (on-chip)
                nc.tensor.transpose(
                    csq_ps[0:1, ts(t, P)], csq_cols[:, t : t + 1], identity[:]
                )
            # csq_sc = (1 + csq/alpha)/128, then broadcast to 128 partitions
            # via a one-off matmul (slow single-partition moving is fine once).
            invp = const_pool.tile([1, 1], f32)
            nc.gpsimd.memset(invp[:], 1.0 / P)
            nc.scalar.activation(
                csq_sc[:],
                csq_ps[0:1, :],
                mybir.ActivationFunctionType.Identity,
                bias=invp[:, 0:1],
                scale=1.0 / P,
            )
            bc_ps = setup_ps1.tile([P, K], f32, tag="bc_ps")
            nc.tensor.matmul(
                bc_ps[:], ones_row[:], csq_sc[:], start=True, stop=True
            )
            nc.scalar.copy(csq_bcast[:], bc_ps[:])

        with (
            tc.tile_pool(name="work", bufs=3) as work_pool,
            tc.tile_pool(name="blk", bufs=4) as blk_pool,
            tc.tile_pool(name="io_out", bufs=3) as out_pool,
            tc.tile_pool(name="tp_ps", bufs=2, space=bass.MemorySpace.PSUM) as tp_ps,
            tc.tile_pool(name="mm_ps", bufs=4, space=bass.MemorySpace.PSUM) as mm_ps,
        ):
            for i in [t for _ in range(reps) for t in range(n_super)]:
                emb_nat = prefetched.pop(i, None)
                if emb_nat is None:
                    emb_nat = in_pool.tile([P, S_BLK, D], f32, tag="emb")
                    nc.sync.dma_start(emb_nat[:], emb_v[i])

                embT = work_pool.tile(
                    [P, S_BLK * D_CHUNKS, P], fp8, tag="embT"
                )
                xsq = work_pool.tile([P, S_BLK], f32, tag="xsq")
                numer = work_pool.tile([P, S_BLK, K], f32, tag="numer")
                out_t = out_pool.tile([P, S_BLK, K], f32, tag="out")

                for a in range(S_BLK):
                    # xsq = ||x||^2/alpha; one block on ACT, rest on DVE
                    sq_scr = blk_pool.tile([P, D], f32, tag="sqscr")
                    if a == 0:
                        nc.scalar.activation(
                            sq_scr[:],
                            emb_nat[:, a, :],
                            mybir.ActivationFunctionType.Square,
                            scale=float(inv_a**0.5),
                            accum_out=xsq[:, a : a + 1],
                        )
                    else:
                        nc.vector.scalar_tensor_tensor(
                            out=sq_scr[:],
                            in0=emb_nat[:, a, :],
                            scalar=inv_a,
                            in1=emb_nat[:, a, :],
                            op0=mybir.AluOpType.mult,
                            op1=mybir.AluOpType.mult,
                            accum_out=xsq[:, a : a + 1],
                        )
                    dst = embT[:, a * D_CHUNKS : (a + 1) * D_CHUNKS, :]
                    tps = tp_ps.tile([P, D_CHUNKS, P], f32, tag="tps")
                    for j in range(D_CHUNKS):
                        nc.tensor.transpose(
                            tps[:, j, :], emb_nat[:, a, ts(j, P)], identity[:]
                        )
                    if a % 2 == 0:
                        nc.scalar.copy(dst, tps[:])
                    else:
                        nc.vector.tensor_copy(out=dst, in_=tps[:])

                for a in range(S_BLK):
                    # PSUM <- (1 + csq/a) - (2/a) x.c
                    ps = mm_ps.tile([P, K], f32, tag="cross")
                    nc.tensor.matmul(
                        ps[:],
                        ones_mat[:],
                        csq_bcast[:],
                        start=True,
                        stop=False,
                    )
                    for j in range(0, D_CHUNKS, 2):
                        nc.tensor.matmul(
                            ps[:],
                            embT[:, a * D_CHUNKS + j : a * D_CHUNKS + j + 2, :],
                            cenT[:, j : j + 2, :],
                            start=False,
                            stop=(j == D_CHUNKS - 2),
                            perf_mode=DR,
                        )

                    if power == 1.0:
                        # numer = 1/(ps + xsq) and rowsum in ONE ACT op
                        rs_a = blk_pool.tile([P, 1], f32, tag="rs")
                        act_recip(
                            numer[:, a, :],
                            ps[:],
                            xsq[:, a : a + 1],
                            rs_a[:],
                        )
                        inv_a_t = blk_pool.tile([P, 1], f32, tag="inv")
                        nc.vector.reciprocal(inv_a_t[:], rs_a[:])
                        nc.vector.tensor_scalar_mul(
                            out_t[:, a, :], numer[:, a, :], inv_a_t[:]
                        )
                    else:
                        denom = blk_pool.tile([P, K], f32, tag="denom")
                        nc.scalar.activation(
                            denom[:],
                            ps[:],
                            mybir.ActivationFunctionType.Identity,
                            bias=xsq[:, a : a + 1],
                            scale=1.0,
                        )
                        lnd = blk_pool.tile([P, K], f32, tag="lnd")
                        rs_a = blk_pool.tile([P, 1], f32, tag="rs")
                        nc.scalar.activation(
                            lnd[:], denom[:], mybir.ActivationFunctionType.Ln
                        )
                        nc.scalar.activation(
                            numer[:, a, :],
                            lnd[:],
                            mybir.ActivationFunctionType.Exp,
                            scale=-power,
                            accum_out=rs_a[:],
                        )
                        inv_a_t = blk_pool.tile([P, 1], f32, tag="inv")
                        nc.vector.reciprocal(inv_a_t[:], rs_a[:])
                        nc.vector.tensor_scalar_mul(
                            out_t[:, a, :], numer[:, a, :], inv_a_t[:]
                        )

                # output trigger on the (otherwise idle) GpSimd queue
                nc.gpsimd.dma_start(out_v[i], out_t[:])


def _build_program(alpha: float):
    """Standalone Bacc program (for CoreSim checks / bench2)."""
    import concourse.bacc as bacc
    import concourse.mybir as mybir
    import concourse.tile as tile

    f32 = mybir.dt.float32
    nc = bacc.Bacc(None, target_bir_lowering=False, debug=False, num_devices=N_CORES)
    emb_d = nc.declare_dram_parameter("embeddings", [NC_ROWS, D], f32, isOutput=False)
    cen_d = nc.declare_dram_parameter("cluster_centers", [K, D], f32, isOutput=False)
    out_d = nc.declare_dram_parameter("cluster_p", [NC_ROWS, K], f32, isOutput=True)
    with tile.TileContext(nc) as tc:
        _emit(nc, tc, emb_d, cen_d, out_d, alpha, NC_ROWS)
    nc.finalize()
    return nc


def _get_jitted(alpha: float):
    key = (
        float(alpha),
        os.environ.get("KBENCH_REPS", "1"),
        os.environ.get("KOPT_TPS", "pair"),
        os.environ.get("KOPT_RECIP", "fused"),
    )
    if key in _CACHE:
        return _CACHE[key]

    import jax
    from jax.experimental.shard_map import shard_map
    from jax.sharding import Mesh, PartitionSpec as PS

    import concourse.mybir as mybir
    import concourse.tile as tile
    from concourse.bass2jax import bass_jit

    f32 = mybir.dt.float32

    def body(nc, emb, cen):
        out_d = nc.dram_tensor(
            "cluster_p", [NC_ROWS, K], f32, kind="ExternalOutput"
        )
        with tile.TileContext(nc) as tc:
            _emit(nc, tc, emb, cen, out_d, float(alpha), NC_ROWS)
        return out_d

    f = bass_jit(body, num_devices=N_CORES)
    mesh = Mesh(np.asarray(jax.devices()[:N_CORES]), ("core",))
    sharded = shard_map(
        f,
        mesh=mesh,
        in_specs=(PS("core"), PS(None)),
        out_specs=PS("core"),
        check_rep=False,
    )
    jitted = jax.jit(sharded)
    _CACHE[key] = (jitted, mesh)
    return _CACHE[key]


def kernel(embeddings, cluster_centers, alpha):
    emb = np.ascontiguousarray(np.asarray(embeddings, dtype=np.float32))
    cen = np.ascontiguousarray(np.asarray(cluster_centers, dtype=np.float32))
    jitted, _ = _get_jitted(float(alpha))
    try:
        out = jitted(emb, cen)
        return np.asarray(out)
    except Exception:
        # transient device hiccups have been observed; retry once
        import time as _time

        _time.sleep(60)
        out = jitted(emb, cen)
        return np.asarray(out)
